# revision 21
# baseline (speedup 1.0000x reference)
"""Trainium2 Bass kernel for a 2-layer MoE GPT (moe_routing).

Model: tok_emb lookup -> 2x [RMSNorm -> causal ALiBi attention -> RMSNorm ->
top-2-of-4 MoE (dense expert compute, sparse combine)] -> RMSNorm -> tied head.

Sharding over 8 NeuronCores:
- attention: head-parallel (2 of 16 heads per core), o_proj partial-summed
  via AllReduce.
- MoE: expert-parallel (expert e=c//2, hidden half c%2 per core), partial
  down_proj outputs AllReduced (routing weights folded in pre-reduce).
- LM head: vocab-parallel (4000 rows per core), concatenated on host.

v2 structure (vs the v1 baseline):
- AllReduces split per 512-token half and pipelined with compute; the
  residual x/8 is folded into each core's AR payload so the AR output IS
  the new residual (DMA'd straight into x, no post-AR adds).
- RMS scaling folded into matmul evacuation for attention qkv and the LM
  head (linear paths), so those matmuls don't wait on the norm.
- Softmax denominator via an augmented ones-column in the value tiles
  (row 64 of the AV accumulation), no separate denominator matmuls.
- Norm squares on the Scalar engine; fused scale-add evacuations
  (scalar_tensor_tensor) for AR payloads.
- Host-side pre-transposed [128, chunk, free] layouts -> single dense
  DMAs for activations, weights and AR buffers.
"""

import sys

sys.path.insert(0, "/opt/trn_rl_repo")

import numpy as np

import concourse.bass as bass  # noqa: E402
import concourse.bacc as bacc  # noqa: E402
import concourse.tile as tile  # noqa: E402
from concourse import mybir  # noqa: E402
from concourse.bass_utils import run_bass_kernel_spmd  # noqa: E402

dt = mybir.dt
F32 = dt.float32
F32R = dt.float32r
AF = mybir.ActivationFunctionType
ALU = mybir.AluOpType

V, D, H, HD, L, E, HID, TOPK, B, T = 32000, 1024, 16, 64, 2, 4, 2048, 2, 1, 1024
EPS = 1e-8
NC_N = 8
VS = V // NC_N            # vocab shard per core (4000)
VC = 500                  # head matmul free-dim chunk (8 * 500 = 4000)
DC = D // 128             # 8 partition chunks of the model dim
HC = 1024 // 128          # 8 hidden chunks of the per-core expert slice
NQ = 2                    # T halves of 512 for matmul free dim
TQ = T // NQ              # 512
TC2 = DC // NQ            # 128-token chunks per half (4)
NEG = -1.0e30

# (nq, kc) score tiles that are not fully causally masked, in emission order
ATT_TILES = [(nq, kc) for nq in range(NQ) for kc in range((nq + 1) * 4)]


def _ap(x):
    return x.ap() if hasattr(x, "ap") else x


def build_nc():
    nc = bacc.Bacc("TRN2", target_bir_lowering=False, debug=False,
                   num_devices=NC_N)

    # ---- DRAM I/O (per-core payloads; same shapes on every core) ----
    # activations / weights pre-transposed host-side to [128, chunk, free]
    x0 = nc.dram_tensor("x0", [128, DC, T], F32, kind="ExternalInput")
    biasP = nc.dram_tensor("biasP", [2 * len(ATT_TILES), 128, TQ], F32,
                           kind="ExternalInput")
    qkvw = nc.dram_tensor("qkvw", [L, 128, DC, 384], F32, kind="ExternalInput")
    ow = nc.dram_tensor("ow", [L, 128, D], F32, kind="ExternalInput")
    routw = nc.dram_tensor("routw", [L, 128, DC, E], F32, kind="ExternalInput")
    gatew = nc.dram_tensor("gatew", [L, 128, DC, 1024], F32, kind="ExternalInput")
    upw = nc.dram_tensor("upw", [L, 128, DC, 1024], F32, kind="ExternalInput")
    downw = nc.dram_tensor("downw", [L, 128, HC, D], F32, kind="ExternalInput")
    selw = nc.dram_tensor("selw", [L, E, 128], F32, kind="ExternalInput")
    headw = nc.dram_tensor("headw", [128, DC, VS], F32, kind="ExternalInput")
    cones = nc.dram_tensor("cones", [128, 128], F32, kind="ExternalInput")
    ident = nc.dram_tensor("ident", [128, 128], F32, kind="ExternalInput")
    logits = nc.dram_tensor("logits", [T, VS], F32, kind="ExternalOutput")

    import os
    repeat = int(os.environ.get("KERNEL_REPEAT", "1"))
    with tile.TileContext(nc) as tc, nc.allow_low_precision(
            reason="float32r tiles store full fp32 bits; PE truncates on read"):
        for _ in range(repeat):
            _emit(nc, tc, x0, biasP, qkvw, ow, routw, gatew, upw, downw, selw,
                  headw, cones, ident, logits)
    nc.compile()
    return nc


def _emit(nc, tc, x0, biasP, qkvw, ow, routw, gatew, upw, downw, selw,
          headw, cones, ident, logits):
    import os
    SKIP_ATTN = os.environ.get("SKIP_ATTN") == "1"
    SKIP_MOE = os.environ.get("SKIP_MOE") == "1"
    SKIP_HEAD = os.environ.get("SKIP_HEAD") == "1"
    SKIP_AR = os.environ.get("SKIP_AR") == "1"
    ctxpools = []
    cm_of = {}

    def pool(name, bufs=1, space="SBUF"):
        p = tc.tile_pool(name=name, bufs=bufs, space=space)
        pm = p.__enter__()
        ctxpools.append(pm)
        cm_of[id(pm)] = p
        return pm

    def close_pool(pm):
        cm_of[id(pm)].__exit__(None, None, None)
        ctxpools.remove(pm)

    outer = pool("outer", bufs=1)
    dram = pool("dram", bufs=1, space="DRAM")

    # constants
    ones_t = outer.tile([128, 128], F32R, name="ones", tag="ones", bufs=1)
    nc.sync.dma_start(out=ones_t[:], in_=_ap(cones)[:].bitcast(F32R))
    id_t = outer.tile([128, 128], F32, name="ident", tag="ident", bufs=1)
    nc.sync.dma_start(out=id_t[:], in_=_ap(ident)[:])

    # residual stream [128, DC, T] fp32 (chunk a holds rows a*128..a*128+127)
    x_all = outer.tile([128, DC, T], F32R, name="x", tag="x", bufs=1)
    nc.sync.dma_start(out=x_all[:], in_=_ap(x0)[:].bitcast(F32R))
    xr = x_all[:]  # matmul-operand view

    # normalized activations for the MoE (fp32 bits, tagged f32r)
    xn_all = outer.tile([128, DC, T], F32R, name="xn", tag="xn", bufs=1)

    # staging tiles for AR payload chunks (triple-buffered; one DMA per
    # 128-row chunk)
    p_stage = pool("stage", bufs=3)

    def ts_(h):
        return slice(h * TQ, (h + 1) * TQ)

    def rstd_half(tag, h, rstdb, rstd_out=None):
        """rstdb[:, :] = broadcast of 1/sqrt(mean_d x^2 + EPS) for half h."""
        ts = ts_(h)
        p_sq = pool(f"sq_{tag}", bufs=1)
        p_ps = pool(f"nps_{tag}", bufs=1, space="PSUM")
        ssq = p_ps.tile([1, TQ], F32, name="ssq", tag="ssq", bufs=1)
        for i in range(DC):
            sq = p_sq.tile([128, TQ], F32R, name="sq", tag="sq", bufs=2)
            nc.scalar.activation(sq[:], x_all[:, i, ts], AF.Square)
            nc.tensor.matmul(ssq[:], ones_t[:, 0:1], sq[:],
                             start=(i == 0), stop=(i == DC - 1))
        tmp = p_sq.tile([1, 2, TQ], F32, name="ntmp", tag="ntmp", bufs=1)
        t0 = tmp[:, 0, :]
        t1 = tmp[:, 1, :]
        nc.vector.tensor_scalar(t0, ssq[:], 1.0 / D, EPS, ALU.mult, ALU.add)
        nc.scalar.activation(t1, t0, AF.Sqrt)
        if rstd_out is None:
            rstd_out = p_sq.tile([1, TQ], F32R, name="rstd", tag="rstd", bufs=1)
        nc.vector.reciprocal(rstd_out[:], t1)
        if rstdb is not None:
            bps = p_ps.tile([128, TQ], F32, name="bps", tag="bps", bufs=1)
            nc.tensor.matmul(bps[:], ones_t[0:1, :], rstd_out[:],
                             start=True, stop=True)
            nc.vector.tensor_copy(rstdb[:], bps[:])
        close_pool(p_ps)
        close_pool(p_sq)

    # ---------------- layers ----------------
    for l in range(L):
        if SKIP_ATTN:
            break
        # ---- attention ----
        p_aw = pool(f"aw{l}", bufs=1)
        qkv_t = p_aw.tile([128, DC, 384], F32R, name="qkvw", tag="qkvw", bufs=1)
        nc.sync.dma_start(out=qkv_t[:], in_=_ap(qkvw)[l].bitcast(F32R))
        ow_h = []
        for h in range(2):
            t = p_aw.tile([64, D], F32R, name=f"ow{h}", tag=f"ow{h}", bufs=1)
            nc.sync.dma_start(out=t[:],
                              in_=_ap(ow)[l][h * 64:(h + 1) * 64, :]
                              .bitcast(F32R))
            ow_h.append(t)

        rstdb_a = [p_aw.tile([128, TQ], F32, name=f"rsb{h}", tag=f"rsb{h}",
                             bufs=1) for h in range(NQ)]
        for h in range(NQ):
            rstd_half(f"a{l}h{h}", h, rstdb_a[h])

        # qkv on raw x; rstd folded into the PSUM evacuation
        qkvT = p_aw.tile([128, 3, T], F32R, name="qkvT", tag="qkvT", bufs=1)
        p_qps = pool(f"qps{l}", bufs=1, space="PSUM")
        for nh in range(NQ):
            ts = ts_(nh)
            for m in range(3):
                ps = p_qps.tile([128, TQ], F32, name="qkv", tag="qkv", bufs=3)
                for i in range(DC):
                    nc.tensor.matmul(ps[:],
                                     qkv_t[:, i, m * 128:(m + 1) * 128],
                                     xr[:, i, ts],
                                     start=(i == 0), stop=(i == DC - 1))
                nc.vector.tensor_tensor(
                    qkvT[:, m, ts], ps[:], rstdb_a[nh][:], ALU.mult)
        qT = qkvT[:, 0, :]
        kT = qkvT[:, 1, :]
        vT = qkvT[:, 2, :]
        # v in natural [token, hd] layout, augmented with a ones column per
        # head (row 64 of the AV output then accumulates the softmax denom)
        v_aug = p_aw.tile([128, DC, 130], F32R, name="vaug", tag="vaug", bufs=1)
        nc.vector.tensor_copy(v_aug[:, :, 64:65], ones_t[:, 0:DC].unsqueeze(2))
        nc.vector.tensor_copy(v_aug[:, :, 129:130], ones_t[:, 0:DC].unsqueeze(2))
        for tcn in range(DC):
            tp = p_qps.tile([128, 128], F32, name="vt", tag="vt", bufs=2)
            nc.tensor.transpose(tp[:],
                                vT[:, tcn * 128:(tcn + 1) * 128].bitcast(F32),
                                id_t[:])
            for h in range(2):
                nc.vector.tensor_copy(v_aug[:, tcn, h * 65:h * 65 + 64],
                                      tp[:, h * 64:(h + 1) * 64])
        close_pool(p_qps)

        yTh = [p_aw.tile([64, T], F32R, name=f"yT{h}", tag=f"yT{h}", bufs=1)
               for h in range(2)]
        p_sps = pool(f"sps{l}", bufs=1, space="PSUM")
        p_sc = pool(f"sc{l}", bufs=1)
        p_ops = pool(f"ops{l}", bufs=1, space="PSUM")
        for nq in range(NQ):
            ts = ts_(nq)
            kcs = [kc for (q, kc) in ATT_TILES if q == nq]
            yps = [p_sps.tile([65, TQ], F32, name=f"yt{h}", tag=f"yt{h}", bufs=1)
                   for h in range(2)]
            for ki, kc in enumerate(kcs):
                for h in range(2):
                    hp = h * 64
                    st = p_sps.tile([128, TQ], F32, name="st", tag="st", bufs=3)
                    nc.tensor.matmul(st[:],
                                     kT[hp:hp + 64, kc * 128:(kc + 1) * 128],
                                     qT[hp:hp + 64, ts],
                                     start=True, stop=True)
                    bti = (h * len(ATT_TILES)
                           + ATT_TILES.index((nq, kc)))
                    bt = p_sc.tile([128, TQ], F32, name="bias", tag="bias", bufs=3)
                    nc.sync.dma_start(out=bt[:], in_=_ap(biasP)[bti])
                    es = p_sc.tile([128, TQ], F32R, name="es", tag="es", bufs=3)
                    nc.vector.tensor_tensor(es[:], st[:], bt[:], ALU.add)
                    nc.scalar.activation(es[:], es[:], AF.Exp)
                    nc.tensor.matmul(yps[h][:, :],
                                     v_aug[:, kc, h * 65:(h + 1) * 65],
                                     es[:],
                                     start=(ki == 0), stop=(ki == len(kcs) - 1))
            # normalize: yTh = y_unnorm * (1/denom) broadcast over rows
            for h in range(2):
                rc = p_sc.tile([1, TQ], F32R, name="rc", tag="rc", bufs=2)
                nc.vector.reciprocal(rc[:], yps[h][64:65, :])
                rps = p_sps.tile([64, TQ], F32, name="rb", tag="rb", bufs=1)
                nc.tensor.matmul(rps[:], ones_t[0:1, 0:64],
                                 rc[:], start=True, stop=True)
                rsb = p_sc.tile([64, TQ], F32, name="rsb", tag="rsb", bufs=2)
                nc.scalar.copy(rsb[:], rps[:])
                nc.vector.tensor_tensor(
                    yTh[h][:, ts], yps[h][0:64, :], rsb[:, :], ALU.mult)

            # o_proj partials for this half; residual x/8 folded in
            arin = dram.tile([128, DC, TQ], F32, name=f"arin_a{l}{nq}",
                             tag=f"arin_a{nq}", bufs=1)
            arout = dram.tile([128, DC, TQ], F32, name=f"arout_a{l}{nq}",
                              tag=f"arout_a{nq}", bufs=1, addr_space="Shared")
            for i in range(DC):
                ps = p_ops.tile([128, TQ], F32, name="o", tag="o", bufs=2)
                for h in range(2):
                    nc.tensor.matmul(ps[:],
                                     ow_h[h][:, i * 128:(i + 1) * 128],
                                     yTh[h][:, ts],
                                     start=(h == 0), stop=(h == 1))
                stg = p_stage.tile([128, TQ], F32, name="stg", tag="stg", bufs=3)
                nc.vector.scalar_tensor_tensor(
                    stg[:], x_all[:, i, ts].bitcast(F32), 1.0 / NC_N, ps[:],
                    ALU.mult, ALU.add)
                nc.sync.dma_start(out=arin[:, i, :], in_=stg[:])
            if not SKIP_AR:
                nc.gpsimd.collective_compute(
                    "AllReduce", ALU.add, replica_groups=[list(range(NC_N))],
                    ins=[arin.opt()], outs=[arout.opt()])
                nc.sync.dma_start(out=x_all[:, :, ts],
                                  in_=arout[:].bitcast(F32R))
        for p in (p_ops, p_sc, p_sps, p_aw):
            close_pool(p)

        if SKIP_MOE:
            continue
        # ---- MoE ----
        p_mw = pool(f"mw{l}", bufs=1)
        p_msc = pool(f"msc{l}", bufs=1)
        rout_t = p_mw.tile([128, DC, E], F32, name="routw", tag="routw", bufs=1)
        nc.sync.dma_start(out=rout_t[:], in_=_ap(routw)[l])
        sel_t = p_mw.tile([E, 128], F32, name="sel", tag="sel", bufs=1)
        nc.sync.dma_start(out=sel_t[:], in_=_ap(selw)[l])
        gate_t = p_mw.tile([128, DC, 1024], F32R, name="gate", tag="gate", bufs=1)
        nc.sync.dma_start(out=gate_t[:], in_=_ap(gatew)[l].bitcast(F32R))
        up_t = p_mw.tile([128, DC, 1024], F32R, name="up", tag="up", bufs=1)
        nc.sync.dma_start(out=up_t[:], in_=_ap(upw)[l].bitcast(F32R))

        for mh in range(NQ):
            ts = ts_(mh)
            rstdb_m = p_msc.tile([128, TQ], F32, name="rsbm", tag="rsbm", bufs=2)
            rstd_half(f"f{l}h{mh}", mh, rstdb_m)
            for i in range(DC):
                nc.vector.tensor_tensor(xn_all[:, i, ts],
                                        x_all[:, i, ts],
                                        rstdb_m[:], ALU.mult)

            # router in true fp32 on token chunks of this half
            p_rps = pool(f"rps{l}{mh}", bufs=1, space="PSUM")
            rlog = p_msc.tile([128, TC2, E], F32, name="rlog", tag="rlog", bufs=2)
            for tj in range(TC2):
                tcn = mh * TC2 + tj
                ps = p_rps.tile([128, E], F32, name="rl", tag="rl", bufs=2)
                for i in range(DC):
                    nc.tensor.matmul(ps[:],
                                     xn_all[:, i, tcn * 128:(tcn + 1) * 128]
                                     .bitcast(F32),
                                     rout_t[:, i, :],
                                     start=(i == 0), stop=(i == DC - 1))
                nc.vector.tensor_copy(rlog[:, tj, :], ps[:])
            # batched top-2 -> combine weights  [128, (tj, e)]
            m1 = p_msc.tile([128, TC2], F32, name="m1", tag="m1", bufs=2)
            nc.vector.tensor_reduce(m1[:], rlog[:], mybir.AxisListType.X, ALU.max)
            eq1 = p_msc.tile([128, TC2, E], F32, name="eq1", tag="eq1", bufs=2)
            nc.vector.tensor_tensor(eq1[:], rlog[:],
                                    m1[:].unsqueeze(2).broadcast_to([128, TC2, E]),
                                    ALU.is_equal)
            msk = p_msc.tile([128, TC2, E], F32, name="msk", tag="msk", bufs=2)
            nc.vector.scalar_tensor_tensor(msk[:], eq1[:], NEG, rlog[:],
                                           ALU.mult, ALU.add)
            m2 = p_msc.tile([128, TC2], F32, name="m2", tag="m2", bufs=2)
            nc.vector.tensor_reduce(m2[:], msk[:], mybir.AxisListType.X, ALU.max)
            eq2 = p_msc.tile([128, TC2, E], F32, name="eq2", tag="eq2", bufs=2)
            nc.vector.tensor_tensor(eq2[:], msk[:],
                                    m2[:].unsqueeze(2).broadcast_to([128, TC2, E]),
                                    ALU.is_equal)
            d12 = p_msc.tile([128, TC2], F32, name="d12", tag="d12", bufs=2)
            nc.vector.tensor_tensor(d12[:], m1[:], m2[:], ALU.subtract)
            w1 = p_msc.tile([128, TC2], F32, name="w1", tag="w1", bufs=2)
            nc.scalar.activation(w1[:], d12[:], AF.Sigmoid)
            w2 = p_msc.tile([128, TC2], F32, name="w2", tag="w2", bufs=2)
            nc.vector.tensor_scalar(w2[:], w1[:], -1.0, 1.0, ALU.mult, ALU.add)
            comb = p_msc.tile([128, TC2, E], F32, name="comb", tag="comb", bufs=2)
            nc.vector.tensor_tensor(comb[:], eq1[:],
                                    w1[:].unsqueeze(2).broadcast_to([128, TC2, E]),
                                    ALU.mult)
            eq2w = p_msc.tile([128, TC2, E], F32, name="eq2w", tag="eq2w", bufs=2)
            nc.vector.tensor_tensor(eq2w[:], eq2[:],
                                    w2[:].unsqueeze(2).broadcast_to([128, TC2, E]),
                                    ALU.mult)
            nc.vector.tensor_tensor(comb[:], comb[:], eq2w[:], ALU.add)
            # per-token scaled combine weight for this core's expert,
            # broadcast over 128 partitions: bc[p, t] = sel.T @ comb_tj.T
            bc = p_msc.tile([128, TQ], F32, name="bc", tag="bc", bufs=2)
            for tj in range(TC2):
                tp = p_rps.tile([E, 128], F32, name="ct", tag="ct", bufs=2)
                nc.tensor.transpose(tp[:], comb[:, tj, :], id_t[:])
                ct = p_msc.tile([E, 128], F32, name="cts", tag="cts", bufs=2)
                nc.vector.tensor_copy(ct[:], tp[:])
                bp = p_rps.tile([128, 128], F32, name="bcp", tag="bcp", bufs=2)
                nc.tensor.matmul(bp[:], sel_t[:], ct[:], start=True, stop=True)
                nc.vector.tensor_copy(bc[:, tj * 128:(tj + 1) * 128], bp[:])
            close_pool(p_rps)

            # experts: gate/up -> silu*up*bc -> down; residual folded at
            # evacuation; AR per half
            arin2 = dram.tile([128, DC, TQ], F32, name=f"arin_m{l}{mh}",
                              tag=f"arin_m{mh}", bufs=1)
            arout2 = dram.tile([128, DC, TQ], F32, name=f"arout_m{l}{mh}",
                               tag=f"arout_m{mh}", bufs=1, addr_space="Shared")
            p_mps = pool(f"mps{l}{mh}", bufs=1, space="PSUM")
            gu_all = p_msc.tile([128, HC, TQ], F32R, name="gu", tag="gu", bufs=1)
            for hc in range(HC):
                gps = p_mps.tile([128, TQ], F32, name="g", tag="g", bufs=2)
                for i in range(DC):
                    nc.tensor.matmul(gps[:], gate_t[:, i, hc * 128:(hc + 1) * 128],
                                     xn_all[:, i, ts],
                                     start=(i == 0), stop=(i == DC - 1))
                gs = p_msc.tile([128, TQ], F32, name="gs", tag="gs", bufs=2)
                nc.scalar.activation(gs[:], gps[:], AF.Silu)
                ups = p_mps.tile([128, TQ], F32, name="u", tag="u", bufs=2)
                for i in range(DC):
                    nc.tensor.matmul(ups[:], up_t[:, i, hc * 128:(hc + 1) * 128],
                                     xn_all[:, i, ts],
                                     start=(i == 0), stop=(i == DC - 1))
                gu = gu_all[:, hc, :]
                nc.vector.tensor_tensor(gu, gs[:], ups[:], ALU.mult)
                nc.vector.tensor_tensor(gu, gu, bc[:, :], ALU.mult)
            for i in range(DC):
                dw = p_mw.tile([128, HC, 128], F32R, name="down", tag="down", bufs=2)
                nc.sync.dma_start(
                    out=dw[:],
                    in_=_ap(downw)[l][:, :, i * 128:(i + 1) * 128].bitcast(F32R))
                dps = p_mps.tile([128, TQ], F32, name="d", tag="d", bufs=2)
                for hc in range(HC):
                    nc.tensor.matmul(dps[:], dw[:, hc, :], gu_all[:, hc, :],
                                     start=(hc == 0), stop=(hc == HC - 1))
                stg = p_stage.tile([128, TQ], F32, name="stg", tag="stg", bufs=3)
                nc.vector.scalar_tensor_tensor(
                    stg[:], x_all[:, i, ts].bitcast(F32), 1.0 / NC_N, dps[:],
                    ALU.mult, ALU.add)
                nc.sync.dma_start(out=arin2[:, i, :], in_=stg[:])
            close_pool(p_mps)
            if not SKIP_AR:
                nc.gpsimd.collective_compute(
                    "AllReduce", ALU.add, replica_groups=[list(range(NC_N))],
                    ins=[arin2.opt()], outs=[arout2.opt()])
                nc.sync.dma_start(out=x_all[:, :, ts],
                                  in_=arout2[:].bitcast(F32R))
        for p in (p_msc, p_mw):
            close_pool(p)

    if SKIP_HEAD:
        for pm in reversed(list(ctxpools)):
            close_pool(pm)
        return
    # ---- final norm + vocab-sharded tied head ----
    # rstd folded into the head evacuation (per-token scale on partitions)
    p_hw = pool("hw", bufs=1)
    p_hps = pool("hps", bufs=1, space="PSUM")
    rstdc = p_hw.tile([128, DC], F32, name="rstdc", tag="rstdc", bufs=1)
    for h in range(NQ):
        rstdb_h = p_hw.tile([128, TQ], F32, name="rsbh", tag="rsbh", bufs=2)
        rstd_half(f"hd{h}", h, rstdb_h)
        # per-token rstd as a column vector per 128-token chunk: transpose
        # a [128, 128] window of the row-broadcast tile; its columns are
        # all the per-token column we need
        for tj in range(TC2):
            tcn = h * TC2 + tj
            cp = p_hps.tile([128, 128], F32, name="rc", tag="rch", bufs=2)
            nc.tensor.transpose(cp[:],
                                rstdb_h[:, tj * 128:(tj + 1) * 128], id_t[:])
            nc.vector.tensor_copy(rstdc[:, tcn:tcn + 1], cp[:, 0:1])
    for vc in range(VS // VC):
        hw = p_hw.tile([128, DC, VC], F32R, name="hw", tag="hw", bufs=2)
        nc.sync.dma_start(
            out=hw[:],
            in_=_ap(headw)[:, :, vc * VC:(vc + 1) * VC].bitcast(F32R))
        for tcn in range(DC):
            ps = p_hps.tile([128, VC], F32, name="h", tag="h", bufs=4)
            for i in range(DC):
                nc.tensor.matmul(ps[:],
                                 xr[:, i, tcn * 128:(tcn + 1) * 128],
                                 hw[:, i, :],
                                 start=(i == 0), stop=(i == DC - 1))
            lg = p_hw.tile([128, VC], F32, name="lg", tag="lg", bufs=4)
            nc.scalar.activation(lg[:], ps[:], AF.Copy,
                                 scale=rstdc[:, tcn:tcn + 1])
            nc.sync.dma_start(
                out=_ap(logits)[tcn * 128:(tcn + 1) * 128,
                                vc * VC:(vc + 1) * VC],
                in_=lg[:])

    for pm in reversed(list(ctxpools)):
        close_pool(pm)


_NC_CACHE = None


def _get_nc():
    global _NC_CACHE
    if _NC_CACHE is None:
        _NC_CACHE = build_nc()
    return _NC_CACHE


def _pmaj(a):
    """[.., D_outer, free] with D_outer = 128*DC -> [.., 128, DC, free]."""
    s = a.shape
    d = s[-2]
    a = a.reshape(*s[:-2], d // 128, 128, s[-1])
    order = list(range(a.ndim))
    order[-3], order[-2] = order[-2], order[-3]
    return np.ascontiguousarray(a.transpose(order))


def make_in_maps(idx, tok_emb, attn_norm_w, q_w, q_b, kv_w, kv_b, o_w, o_b,
                 ffn_norm_w, router_w, gate_w, up_w, down_w, lnf_w):
    """Host-side sharding: build the per-core input dicts."""
    f32 = np.float32
    idx = np.asarray(idx)
    tok_emb = np.asarray(tok_emb, f32)
    x0T = np.ascontiguousarray(tok_emb[idx[0]].T)  # [D, T]
    x0 = _pmaj(x0T)

    qw = np.asarray(q_w, f32).reshape(L, D, H, HD)
    kvw = np.asarray(kv_w, f32).reshape(L, D, 2, H, HD)
    owf = np.asarray(o_w, f32).reshape(L, H, HD, D)
    anw = np.asarray(attn_norm_w, f32)
    fnw = np.asarray(ffn_norm_w, f32)
    rw = np.asarray(router_w, f32)
    gw = np.asarray(gate_w, f32)
    uw = np.asarray(up_w, f32)
    dw = np.asarray(down_w, f32)
    lnf = np.asarray(lnf_w, f32)

    cones = np.ones((128, 128), f32)
    ident = np.eye(128, dtype=f32)

    in_maps = []
    for c in range(NC_N):
        h0 = 2 * c
        e_core, hh = c // 2, c % 2
        # attention bias tiles (alibi + causal), valid tiles only
        nbt = len(ATT_TILES)
        biasP = np.empty((2 * nbt, 128, TQ), f32)
        for hi in range(2):
            slope = (h0 + hi + 1) / H
            for ti, (nq, kc) in enumerate(ATT_TILES):
                k = kc * 128 + np.arange(128, dtype=f32)[:, None]
                q = (nq * TQ + np.arange(TQ, dtype=f32))[None, :]
                b = slope * (k - q)
                b[k > q] = NEG
                biasP[hi * nbt + ti] = b
        # qkv weights: attn_norm folded in, q scaled by 1/sqrt(HD)
        qkvw = np.empty((L, D, 384), f32)
        for l in range(L):
            sc = anw[l][:, None]
            qkvw[l, :, 0:128] = (
                qw[l][:, h0:h0 + 2].reshape(D, 128) * sc / np.sqrt(HD))
            qkvw[l, :, 128:256] = kvw[l][:, 0, h0:h0 + 2].reshape(D, 128) * sc
            qkvw[l, :, 256:384] = kvw[l][:, 1, h0:h0 + 2].reshape(D, 128) * sc
        qkvw = _pmaj(qkvw)
        ow_c = np.ascontiguousarray(owf[:, h0:h0 + 2].reshape(L, 128, D))
        routw = _pmaj(rw * fnw[:, :, None])
        gatew = _pmaj(np.ascontiguousarray(
            gw[:, e_core, :, hh * 1024:(hh + 1) * 1024] * fnw[:, :, None]))
        upw = _pmaj(np.ascontiguousarray(
            uw[:, e_core, :, hh * 1024:(hh + 1) * 1024] * fnw[:, :, None]))
        downw = _pmaj(np.ascontiguousarray(dw[:, e_core, hh * 1024:(hh + 1) * 1024]))
        selw = np.zeros((L, E, 128), f32)
        for l in range(L):
            selw[l, e_core, :] = 1.0 / np.sqrt(l + 1)
        headw = _pmaj(np.ascontiguousarray(
            (tok_emb[c * VS:(c + 1) * VS] * lnf[None, :]).T))
        in_maps.append(dict(
            x0=x0, biasP=biasP, qkvw=qkvw, ow=ow_c, routw=routw,
            gatew=gatew, upw=upw, downw=downw, selw=selw, headw=headw,
            cones=cones, ident=ident))
    return in_maps


def kernel(**inputs):
    nc = _get_nc()
    in_maps = make_in_maps(**inputs)
    res = run_bass_kernel_spmd(nc, in_maps, list(range(NC_N)))
    logits = np.concatenate([res.results[c]["logits"] for c in range(NC_N)],
                            axis=1)
    return logits.reshape(B, T, V)


# revision 22
# speedup vs baseline: 1.0586x; 1.0586x over previous
"""Trainium2 Bass kernel for a 2-layer MoE GPT (moe_routing).

Model: tok_emb lookup -> 2x [RMSNorm -> causal ALiBi attention -> RMSNorm ->
top-2-of-4 MoE (dense expert compute, sparse combine)] -> RMSNorm -> tied head.

Sharding over 8 NeuronCores:
- attention: head-parallel (2 of 16 heads per core), o_proj partial-summed
  via AllReduce.
- MoE: expert-parallel (expert e=c//2, hidden half c%2 per core), partial
  down_proj outputs AllReduced (routing weights folded in pre-reduce).
- LM head: vocab-parallel (4000 rows per core), concatenated on host.

v2 structure (vs the v1 baseline):
- AllReduces split per 512-token half and pipelined with compute; the
  residual x/8 is folded into each core's AR payload so the AR output IS
  the new residual (DMA'd straight into x, no post-AR adds).
- RMS scaling folded into matmul evacuation for attention qkv and the LM
  head (linear paths), so those matmuls don't wait on the norm.
- Softmax denominator via an augmented ones-column in the value tiles
  (row 64 of the AV accumulation), no separate denominator matmuls.
- Norm squares on the Scalar engine; fused scale-add evacuations
  (scalar_tensor_tensor) for AR payloads.
- Host-side pre-transposed [128, chunk, free] layouts -> single dense
  DMAs for activations, weights and AR buffers.
"""

import sys

sys.path.insert(0, "/opt/trn_rl_repo")

import numpy as np

import concourse.bass as bass  # noqa: E402
import concourse.bacc as bacc  # noqa: E402
import concourse.tile as tile  # noqa: E402
from concourse import mybir  # noqa: E402
from concourse.bass_utils import run_bass_kernel_spmd  # noqa: E402

dt = mybir.dt
F32 = dt.float32
F32R = dt.float32r
AF = mybir.ActivationFunctionType
ALU = mybir.AluOpType

V, D, H, HD, L, E, HID, TOPK, B, T = 32000, 1024, 16, 64, 2, 4, 2048, 2, 1, 1024
EPS = 1e-8
NC_N = 8
VS = V // NC_N            # vocab shard per core (4000)
VC = 500                  # head matmul free-dim chunk (8 * 500 = 4000)
DC = D // 128             # 8 partition chunks of the model dim
HC = 1024 // 128          # 8 hidden chunks of the per-core expert slice
NQ = 2                    # T halves of 512 for matmul free dim
TQ = T // NQ              # 512
TC2 = DC // NQ            # 128-token chunks per half (4)
NEG = -1.0e30

# (nq, kc) score tiles that are not fully causally masked, in emission order
ATT_TILES = [(nq, kc) for nq in range(NQ) for kc in range((nq + 1) * 4)]


def _ap(x):
    return x.ap() if hasattr(x, "ap") else x


def build_nc():
    nc = bacc.Bacc("TRN2", target_bir_lowering=False, debug=False,
                   num_devices=NC_N)

    # ---- DRAM I/O (per-core payloads; same shapes on every core) ----
    # activations / weights pre-transposed host-side to [128, chunk, free]
    x0 = nc.dram_tensor("x0", [128, DC, T], F32, kind="ExternalInput")
    biasP = nc.dram_tensor("biasP", [2 * len(ATT_TILES), 128, TQ], F32,
                           kind="ExternalInput")
    qkvw = nc.dram_tensor("qkvw", [L, 128, DC, 384], F32, kind="ExternalInput")
    ow = nc.dram_tensor("ow", [L, 128, D], F32, kind="ExternalInput")
    routw = nc.dram_tensor("routw", [L, 128, DC, E], F32, kind="ExternalInput")
    gatew = nc.dram_tensor("gatew", [L, 128, DC, 1024], F32, kind="ExternalInput")
    upw = nc.dram_tensor("upw", [L, 128, DC, 1024], F32, kind="ExternalInput")
    downw = nc.dram_tensor("downw", [L, 128, HC, D], F32, kind="ExternalInput")
    selw = nc.dram_tensor("selw", [L, E, 128], F32, kind="ExternalInput")
    headw = nc.dram_tensor("headw", [128, DC, VS], F32, kind="ExternalInput")
    cones = nc.dram_tensor("cones", [128, 128], F32, kind="ExternalInput")
    ident = nc.dram_tensor("ident", [128, 128], F32, kind="ExternalInput")
    logits = nc.dram_tensor("logits", [T, VS], F32, kind="ExternalOutput")

    import os
    repeat = int(os.environ.get("KERNEL_REPEAT", "1"))
    with tile.TileContext(nc) as tc, nc.allow_low_precision(
            reason="float32r tiles store full fp32 bits; PE truncates on read"):
        for _ in range(repeat):
            _emit(nc, tc, x0, biasP, qkvw, ow, routw, gatew, upw, downw, selw,
                  headw, cones, ident, logits)
    nc.compile()
    return nc


def _emit(nc, tc, x0, biasP, qkvw, ow, routw, gatew, upw, downw, selw,
          headw, cones, ident, logits):
    import os
    SKIP_ATTN = os.environ.get("SKIP_ATTN") == "1"
    SKIP_MOE = os.environ.get("SKIP_MOE") == "1"
    SKIP_HEAD = os.environ.get("SKIP_HEAD") == "1"
    SKIP_AR = os.environ.get("SKIP_AR") == "1"
    ctxpools = []
    cm_of = {}

    def pool(name, bufs=1, space="SBUF"):
        p = tc.tile_pool(name=name, bufs=bufs, space=space)
        pm = p.__enter__()
        ctxpools.append(pm)
        cm_of[id(pm)] = p
        return pm

    def close_pool(pm):
        cm_of[id(pm)].__exit__(None, None, None)
        ctxpools.remove(pm)

    outer = pool("outer", bufs=1)
    dram = pool("dram", bufs=1, space="DRAM")

    # constants
    ones_t = outer.tile([128, 128], F32R, name="ones", tag="ones", bufs=1)
    nc.sync.dma_start(out=ones_t[:], in_=_ap(cones)[:].bitcast(F32R))
    id_t = outer.tile([128, 128], F32, name="ident", tag="ident", bufs=1)
    nc.sync.dma_start(out=id_t[:], in_=_ap(ident)[:])

    # residual stream [128, DC, T] fp32 (chunk a holds rows a*128..a*128+127)
    x_all = outer.tile([128, DC, T], F32R, name="x", tag="x", bufs=1)
    nc.sync.dma_start(out=x_all[:], in_=_ap(x0)[:].bitcast(F32R))
    xr = x_all[:]  # matmul-operand view

    # normalized activations for the MoE (fp32 bits, tagged f32r)
    xn_all = outer.tile([128, DC, T], F32R, name="xn", tag="xn", bufs=1)

    # staging tiles for AR payload chunks (triple-buffered; one DMA per
    # 128-row chunk)
    p_stage = pool("stage", bufs=3)

    def ts_(h):
        return slice(h * TQ, (h + 1) * TQ)

    def rstd_half(tag, h, rstdb, rstd_out=None):
        """rstdb[:, :] = broadcast of 1/sqrt(mean_d x^2 + EPS) for half h."""
        ts = ts_(h)
        p_sq = pool(f"sq_{tag}", bufs=1)
        p_ps = pool(f"nps_{tag}", bufs=1, space="PSUM")
        ssq = p_ps.tile([1, TQ], F32, name="ssq", tag="ssq", bufs=1)
        for i in range(DC):
            sq = p_sq.tile([128, TQ], F32R, name="sq", tag="sq", bufs=2)
            nc.scalar.activation(sq[:], x_all[:, i, ts], AF.Square)
            nc.tensor.matmul(ssq[:], ones_t[:, 0:1], sq[:],
                             start=(i == 0), stop=(i == DC - 1))
        tmp = p_sq.tile([1, 2, TQ], F32, name="ntmp", tag="ntmp", bufs=1)
        t0 = tmp[:, 0, :]
        t1 = tmp[:, 1, :]
        nc.vector.tensor_scalar(t0, ssq[:], 1.0 / D, EPS, ALU.mult, ALU.add)
        nc.scalar.activation(t1, t0, AF.Sqrt)
        if rstd_out is None:
            rstd_out = p_sq.tile([1, TQ], F32R, name="rstd", tag="rstd", bufs=1)
        nc.vector.reciprocal(rstd_out[:], t1)
        if rstdb is not None:
            bps = p_ps.tile([128, TQ], F32, name="bps", tag="bps", bufs=1)
            nc.tensor.matmul(bps[:], ones_t[0:1, :], rstd_out[:],
                             start=True, stop=True)
            nc.vector.tensor_copy(rstdb[:], bps[:])
        close_pool(p_ps)
        close_pool(p_sq)

    # ---------------- layers ----------------
    for l in range(L):
        if SKIP_ATTN:
            break
        # ---- attention ----
        p_aw = pool(f"aw{l}", bufs=1)
        qkv_t = p_aw.tile([128, DC, 384], F32R, name="qkvw", tag="qkvw", bufs=1)
        nc.sync.dma_start(out=qkv_t[:], in_=_ap(qkvw)[l].bitcast(F32R))
        ow_h = []
        for h in range(2):
            t = p_aw.tile([64, D], F32R, name=f"ow{h}", tag=f"ow{h}", bufs=1)
            nc.sync.dma_start(out=t[:],
                              in_=_ap(ow)[l][h * 64:(h + 1) * 64, :]
                              .bitcast(F32R))
            ow_h.append(t)

        rstdb_a = [p_aw.tile([128, TQ], F32, name=f"rsb{h}", tag=f"rsb{h}",
                             bufs=1) for h in range(NQ)]
        for h in range(NQ):
            rstd_half(f"a{l}h{h}", h, rstdb_a[h])

        # qkv on raw x; rstd folded into the PSUM evacuation
        qkvT = p_aw.tile([128, 3, T], F32R, name="qkvT", tag="qkvT", bufs=1)
        p_qps = pool(f"qps{l}", bufs=1, space="PSUM")
        for nh in range(NQ):
            ts = ts_(nh)
            for m in range(3):
                ps = p_qps.tile([128, TQ], F32, name="qkv", tag="qkv", bufs=3)
                for i in range(DC):
                    nc.tensor.matmul(ps[:],
                                     qkv_t[:, i, m * 128:(m + 1) * 128],
                                     xr[:, i, ts],
                                     start=(i == 0), stop=(i == DC - 1))
                nc.vector.tensor_tensor(
                    qkvT[:, m, ts], ps[:], rstdb_a[nh][:], ALU.mult)
        qT = qkvT[:, 0, :]
        kT = qkvT[:, 1, :]
        vT = qkvT[:, 2, :]
        # v in natural [token, hd] layout, augmented with a ones column per
        # head (row 64 of the AV output then accumulates the softmax denom)
        v_aug = p_aw.tile([128, DC, 130], F32R, name="vaug", tag="vaug", bufs=1)
        nc.vector.tensor_copy(v_aug[:, :, 64:65], ones_t[:, 0:DC].unsqueeze(2))
        nc.vector.tensor_copy(v_aug[:, :, 129:130], ones_t[:, 0:DC].unsqueeze(2))
        for tcn in range(DC):
            tp = p_qps.tile([128, 128], F32, name="vt", tag="vt", bufs=2)
            nc.tensor.transpose(tp[:],
                                vT[:, tcn * 128:(tcn + 1) * 128].bitcast(F32),
                                id_t[:])
            for h in range(2):
                nc.vector.tensor_copy(v_aug[:, tcn, h * 65:h * 65 + 64],
                                      tp[:, h * 64:(h + 1) * 64])
        close_pool(p_qps)

        yTh = [p_aw.tile([64, T], F32R, name=f"yT{h}", tag=f"yT{h}", bufs=1)
               for h in range(2)]
        p_sps = pool(f"sps{l}", bufs=1, space="PSUM")
        p_sc = pool(f"sc{l}", bufs=1)
        p_ops = pool(f"ops{l}", bufs=1, space="PSUM")
        for nq in range(NQ):
            ts = ts_(nq)
            kcs = [kc for (q, kc) in ATT_TILES if q == nq]
            yps = [p_sps.tile([65, TQ], F32, name=f"yt{h}", tag=f"yt{h}", bufs=1)
                   for h in range(2)]
            for ki, kc in enumerate(kcs):
                for h in range(2):
                    hp = h * 64
                    st = p_sps.tile([128, TQ], F32, name="st", tag="st", bufs=3)
                    nc.tensor.matmul(st[:],
                                     kT[hp:hp + 64, kc * 128:(kc + 1) * 128],
                                     qT[hp:hp + 64, ts],
                                     start=True, stop=True)
                    bti = (h * len(ATT_TILES)
                           + ATT_TILES.index((nq, kc)))
                    bt = p_sc.tile([128, TQ], F32, name="bias", tag="bias", bufs=3)
                    nc.sync.dma_start(out=bt[:], in_=_ap(biasP)[bti])
                    es = p_sc.tile([128, TQ], F32R, name="es", tag="es", bufs=3)
                    nc.vector.tensor_tensor(es[:], st[:], bt[:], ALU.add)
                    nc.scalar.activation(es[:], es[:], AF.Exp)
                    nc.tensor.matmul(yps[h][:, :],
                                     v_aug[:, kc, h * 65:(h + 1) * 65],
                                     es[:],
                                     start=(ki == 0), stop=(ki == len(kcs) - 1))
            # normalize: yTh = y_unnorm * (1/denom) broadcast over rows
            for h in range(2):
                rc = p_sc.tile([1, TQ], F32R, name="rc", tag="rc", bufs=2)
                nc.vector.reciprocal(rc[:], yps[h][64:65, :])
                rps = p_sps.tile([64, TQ], F32, name="rb", tag="rb", bufs=1)
                nc.tensor.matmul(rps[:], ones_t[0:1, 0:64],
                                 rc[:], start=True, stop=True)
                rsb = p_sc.tile([64, TQ], F32, name="rsb", tag="rsb", bufs=2)
                nc.scalar.copy(rsb[:], rps[:])
                nc.vector.tensor_tensor(
                    yTh[h][:, ts], yps[h][0:64, :], rsb[:, :], ALU.mult)

            # o_proj partials for this half; residual x/8 folded in
            arin = dram.tile([128, DC, TQ], F32, name=f"arin_a{l}{nq}",
                             tag=f"arin_a{nq}", bufs=1)
            arout = dram.tile([128, DC, TQ], F32, name=f"arout_a{l}{nq}",
                              tag=f"arout_a{nq}", bufs=1, addr_space="Shared")
            for i in range(DC):
                ps = p_ops.tile([128, TQ], F32, name="o", tag="o", bufs=2)
                for h in range(2):
                    nc.tensor.matmul(ps[:],
                                     ow_h[h][:, i * 128:(i + 1) * 128],
                                     yTh[h][:, ts],
                                     start=(h == 0), stop=(h == 1))
                stg = p_stage.tile([128, TQ], F32, name="stg", tag="stg", bufs=3)
                nc.vector.scalar_tensor_tensor(
                    stg[:], x_all[:, i, ts].bitcast(F32), 1.0 / NC_N, ps[:],
                    ALU.mult, ALU.add)
                nc.sync.dma_start(out=arin[:, i, :], in_=stg[:])
            if not SKIP_AR:
                nc.gpsimd.collective_compute(
                    "AllReduce", ALU.add, replica_groups=[list(range(NC_N))],
                    ins=[arin.opt()], outs=[arout.opt()])
                # ACT-ring DMA: waits on the collective sem without
                # head-of-line-blocking the SP DMA FIFO
                nc.scalar.dma_start(out=x_all[:, :, ts],
                                    in_=arout[:].bitcast(F32R))
        for p in (p_ops, p_sc, p_sps, p_aw):
            close_pool(p)

        if SKIP_MOE:
            continue
        # ---- MoE ----
        p_mw = pool(f"mw{l}", bufs=1)
        p_msc = pool(f"msc{l}", bufs=1)
        rout_t = p_mw.tile([128, DC, E], F32, name="routw", tag="routw", bufs=1)
        nc.sync.dma_start(out=rout_t[:], in_=_ap(routw)[l])
        sel_t = p_mw.tile([E, 128], F32, name="sel", tag="sel", bufs=1)
        nc.sync.dma_start(out=sel_t[:], in_=_ap(selw)[l])
        gate_t = p_mw.tile([128, DC, 1024], F32R, name="gate", tag="gate", bufs=1)
        nc.sync.dma_start(out=gate_t[:], in_=_ap(gatew)[l].bitcast(F32R))
        up_t = p_mw.tile([128, DC, 1024], F32R, name="up", tag="up", bufs=1)
        nc.sync.dma_start(out=up_t[:], in_=_ap(upw)[l].bitcast(F32R))

        for mh in range(NQ):
            ts = ts_(mh)
            rstdb_m = p_msc.tile([128, TQ], F32, name="rsbm", tag="rsbm", bufs=2)
            rstd_half(f"f{l}h{mh}", mh, rstdb_m)
            for i in range(DC):
                nc.vector.tensor_tensor(xn_all[:, i, ts],
                                        x_all[:, i, ts],
                                        rstdb_m[:], ALU.mult)

            # router in true fp32 on token chunks of this half
            p_rps = pool(f"rps{l}{mh}", bufs=1, space="PSUM")
            rlog = p_msc.tile([128, TC2, E], F32, name="rlog", tag="rlog", bufs=2)
            for tj in range(TC2):
                tcn = mh * TC2 + tj
                ps = p_rps.tile([128, E], F32, name="rl", tag="rl", bufs=2)
                for i in range(DC):
                    nc.tensor.matmul(ps[:],
                                     xn_all[:, i, tcn * 128:(tcn + 1) * 128]
                                     .bitcast(F32),
                                     rout_t[:, i, :],
                                     start=(i == 0), stop=(i == DC - 1))
                nc.vector.tensor_copy(rlog[:, tj, :], ps[:])
            # batched top-2 -> combine weights  [128, (tj, e)]
            m1 = p_msc.tile([128, TC2], F32, name="m1", tag="m1", bufs=2)
            nc.vector.tensor_reduce(m1[:], rlog[:], mybir.AxisListType.X, ALU.max)
            eq1 = p_msc.tile([128, TC2, E], F32, name="eq1", tag="eq1", bufs=2)
            nc.vector.tensor_tensor(eq1[:], rlog[:],
                                    m1[:].unsqueeze(2).broadcast_to([128, TC2, E]),
                                    ALU.is_equal)
            msk = p_msc.tile([128, TC2, E], F32, name="msk", tag="msk", bufs=2)
            nc.vector.scalar_tensor_tensor(msk[:], eq1[:], NEG, rlog[:],
                                           ALU.mult, ALU.add)
            m2 = p_msc.tile([128, TC2], F32, name="m2", tag="m2", bufs=2)
            nc.vector.tensor_reduce(m2[:], msk[:], mybir.AxisListType.X, ALU.max)
            eq2 = p_msc.tile([128, TC2, E], F32, name="eq2", tag="eq2", bufs=2)
            nc.vector.tensor_tensor(eq2[:], msk[:],
                                    m2[:].unsqueeze(2).broadcast_to([128, TC2, E]),
                                    ALU.is_equal)
            d12 = p_msc.tile([128, TC2], F32, name="d12", tag="d12", bufs=2)
            nc.vector.tensor_tensor(d12[:], m1[:], m2[:], ALU.subtract)
            w1 = p_msc.tile([128, TC2], F32, name="w1", tag="w1", bufs=2)
            nc.scalar.activation(w1[:], d12[:], AF.Sigmoid)
            w2 = p_msc.tile([128, TC2], F32, name="w2", tag="w2", bufs=2)
            nc.vector.tensor_scalar(w2[:], w1[:], -1.0, 1.0, ALU.mult, ALU.add)
            comb = p_msc.tile([128, TC2, E], F32, name="comb", tag="comb", bufs=2)
            nc.vector.tensor_tensor(comb[:], eq1[:],
                                    w1[:].unsqueeze(2).broadcast_to([128, TC2, E]),
                                    ALU.mult)
            eq2w = p_msc.tile([128, TC2, E], F32, name="eq2w", tag="eq2w", bufs=2)
            nc.vector.tensor_tensor(eq2w[:], eq2[:],
                                    w2[:].unsqueeze(2).broadcast_to([128, TC2, E]),
                                    ALU.mult)
            nc.vector.tensor_tensor(comb[:], comb[:], eq2w[:], ALU.add)
            # per-token scaled combine weight for this core's expert,
            # broadcast over 128 partitions: bc[p, t] = sel.T @ comb_tj.T
            bc = p_msc.tile([128, TQ], F32, name="bc", tag="bc", bufs=2)
            for tj in range(TC2):
                tp = p_rps.tile([E, 128], F32, name="ct", tag="ct", bufs=2)
                nc.tensor.transpose(tp[:], comb[:, tj, :], id_t[:])
                ct = p_msc.tile([E, 128], F32, name="cts", tag="cts", bufs=2)
                nc.vector.tensor_copy(ct[:], tp[:])
                bp = p_rps.tile([128, 128], F32, name="bcp", tag="bcp", bufs=2)
                nc.tensor.matmul(bp[:], sel_t[:], ct[:], start=True, stop=True)
                nc.vector.tensor_copy(bc[:, tj * 128:(tj + 1) * 128], bp[:])
            close_pool(p_rps)

            # experts: gate/up -> silu*up*bc -> down; residual folded at
            # evacuation; AR per half
            arin2 = dram.tile([128, DC, TQ], F32, name=f"arin_m{l}{mh}",
                              tag=f"arin_m{mh}", bufs=1)
            arout2 = dram.tile([128, DC, TQ], F32, name=f"arout_m{l}{mh}",
                               tag=f"arout_m{mh}", bufs=1, addr_space="Shared")
            p_mps = pool(f"mps{l}{mh}", bufs=1, space="PSUM")
            gu_all = p_msc.tile([128, HC, TQ], F32R, name="gu", tag="gu", bufs=1)
            for hc in range(HC):
                gps = p_mps.tile([128, TQ], F32, name="g", tag="g", bufs=2)
                for i in range(DC):
                    nc.tensor.matmul(gps[:], gate_t[:, i, hc * 128:(hc + 1) * 128],
                                     xn_all[:, i, ts],
                                     start=(i == 0), stop=(i == DC - 1))
                gs = p_msc.tile([128, TQ], F32, name="gs", tag="gs", bufs=2)
                nc.scalar.activation(gs[:], gps[:], AF.Silu)
                ups = p_mps.tile([128, TQ], F32, name="u", tag="u", bufs=2)
                for i in range(DC):
                    nc.tensor.matmul(ups[:], up_t[:, i, hc * 128:(hc + 1) * 128],
                                     xn_all[:, i, ts],
                                     start=(i == 0), stop=(i == DC - 1))
                gu = gu_all[:, hc, :]
                nc.vector.tensor_tensor(gu, gs[:], ups[:], ALU.mult)
                nc.vector.tensor_tensor(gu, gu, bc[:, :], ALU.mult)
            for i in range(DC):
                dw = p_mw.tile([128, HC, 128], F32R, name="down", tag="down", bufs=2)
                nc.sync.dma_start(
                    out=dw[:],
                    in_=_ap(downw)[l][:, :, i * 128:(i + 1) * 128].bitcast(F32R))
                dps = p_mps.tile([128, TQ], F32, name="d", tag="d", bufs=2)
                for hc in range(HC):
                    nc.tensor.matmul(dps[:], dw[:, hc, :], gu_all[:, hc, :],
                                     start=(hc == 0), stop=(hc == HC - 1))
                stg = p_stage.tile([128, TQ], F32, name="stg", tag="stg", bufs=3)
                nc.vector.scalar_tensor_tensor(
                    stg[:], x_all[:, i, ts].bitcast(F32), 1.0 / NC_N, dps[:],
                    ALU.mult, ALU.add)
                nc.sync.dma_start(out=arin2[:, i, :], in_=stg[:])
            close_pool(p_mps)
            if not SKIP_AR:
                nc.gpsimd.collective_compute(
                    "AllReduce", ALU.add, replica_groups=[list(range(NC_N))],
                    ins=[arin2.opt()], outs=[arout2.opt()])
                nc.scalar.dma_start(out=x_all[:, :, ts],
                                    in_=arout2[:].bitcast(F32R))
        for p in (p_msc, p_mw):
            close_pool(p)

    if SKIP_HEAD:
        for pm in reversed(list(ctxpools)):
            close_pool(pm)
        return
    # ---- final norm + vocab-sharded tied head ----
    # rstd folded into the head evacuation (per-token scale on partitions)
    p_hw = pool("hw", bufs=1)
    p_hps = pool("hps", bufs=1, space="PSUM")
    rstdc = p_hw.tile([128, DC], F32, name="rstdc", tag="rstdc", bufs=1)
    for h in range(NQ):
        rstdb_h = p_hw.tile([128, TQ], F32, name="rsbh", tag="rsbh", bufs=2)
        rstd_half(f"hd{h}", h, rstdb_h)
        # per-token rstd as a column vector per 128-token chunk: transpose
        # a [128, 128] window of the row-broadcast tile; its columns are
        # all the per-token column we need
        for tj in range(TC2):
            tcn = h * TC2 + tj
            cp = p_hps.tile([128, 128], F32, name="rc", tag="rch", bufs=2)
            nc.tensor.transpose(cp[:],
                                rstdb_h[:, tj * 128:(tj + 1) * 128], id_t[:])
            nc.vector.tensor_copy(rstdc[:, tcn:tcn + 1], cp[:, 0:1])
    for vc in range(VS // VC):
        hw = p_hw.tile([128, DC, VC], F32R, name="hw", tag="hw", bufs=2)
        nc.sync.dma_start(
            out=hw[:],
            in_=_ap(headw)[:, :, vc * VC:(vc + 1) * VC].bitcast(F32R))
        for tcn in range(DC):
            ps = p_hps.tile([128, VC], F32, name="h", tag="h", bufs=4)
            for i in range(DC):
                nc.tensor.matmul(ps[:],
                                 xr[:, i, tcn * 128:(tcn + 1) * 128],
                                 hw[:, i, :],
                                 start=(i == 0), stop=(i == DC - 1))
            lg = p_hw.tile([128, VC], F32, name="lg", tag="lg", bufs=4)
            nc.scalar.activation(lg[:], ps[:], AF.Copy,
                                 scale=rstdc[:, tcn:tcn + 1])
            nc.sync.dma_start(
                out=_ap(logits)[tcn * 128:(tcn + 1) * 128,
                                vc * VC:(vc + 1) * VC],
                in_=lg[:])

    for pm in reversed(list(ctxpools)):
        close_pool(pm)


_NC_CACHE = None


def _get_nc():
    global _NC_CACHE
    if _NC_CACHE is None:
        _NC_CACHE = build_nc()
    return _NC_CACHE


def _pmaj(a):
    """[.., D_outer, free] with D_outer = 128*DC -> [.., 128, DC, free]."""
    s = a.shape
    d = s[-2]
    a = a.reshape(*s[:-2], d // 128, 128, s[-1])
    order = list(range(a.ndim))
    order[-3], order[-2] = order[-2], order[-3]
    return np.ascontiguousarray(a.transpose(order))


def make_in_maps(idx, tok_emb, attn_norm_w, q_w, q_b, kv_w, kv_b, o_w, o_b,
                 ffn_norm_w, router_w, gate_w, up_w, down_w, lnf_w):
    """Host-side sharding: build the per-core input dicts."""
    f32 = np.float32
    idx = np.asarray(idx)
    tok_emb = np.asarray(tok_emb, f32)
    x0T = np.ascontiguousarray(tok_emb[idx[0]].T)  # [D, T]
    x0 = _pmaj(x0T)

    qw = np.asarray(q_w, f32).reshape(L, D, H, HD)
    kvw = np.asarray(kv_w, f32).reshape(L, D, 2, H, HD)
    owf = np.asarray(o_w, f32).reshape(L, H, HD, D)
    anw = np.asarray(attn_norm_w, f32)
    fnw = np.asarray(ffn_norm_w, f32)
    rw = np.asarray(router_w, f32)
    gw = np.asarray(gate_w, f32)
    uw = np.asarray(up_w, f32)
    dw = np.asarray(down_w, f32)
    lnf = np.asarray(lnf_w, f32)

    cones = np.ones((128, 128), f32)
    ident = np.eye(128, dtype=f32)

    in_maps = []
    for c in range(NC_N):
        h0 = 2 * c
        e_core, hh = c // 2, c % 2
        # attention bias tiles (alibi + causal), valid tiles only
        nbt = len(ATT_TILES)
        biasP = np.empty((2 * nbt, 128, TQ), f32)
        for hi in range(2):
            slope = (h0 + hi + 1) / H
            for ti, (nq, kc) in enumerate(ATT_TILES):
                k = kc * 128 + np.arange(128, dtype=f32)[:, None]
                q = (nq * TQ + np.arange(TQ, dtype=f32))[None, :]
                b = slope * (k - q)
                b[k > q] = NEG
                biasP[hi * nbt + ti] = b
        # qkv weights: attn_norm folded in, q scaled by 1/sqrt(HD)
        qkvw = np.empty((L, D, 384), f32)
        for l in range(L):
            sc = anw[l][:, None]
            qkvw[l, :, 0:128] = (
                qw[l][:, h0:h0 + 2].reshape(D, 128) * sc / np.sqrt(HD))
            qkvw[l, :, 128:256] = kvw[l][:, 0, h0:h0 + 2].reshape(D, 128) * sc
            qkvw[l, :, 256:384] = kvw[l][:, 1, h0:h0 + 2].reshape(D, 128) * sc
        qkvw = _pmaj(qkvw)
        ow_c = np.ascontiguousarray(owf[:, h0:h0 + 2].reshape(L, 128, D))
        routw = _pmaj(rw * fnw[:, :, None])
        gatew = _pmaj(np.ascontiguousarray(
            gw[:, e_core, :, hh * 1024:(hh + 1) * 1024] * fnw[:, :, None]))
        upw = _pmaj(np.ascontiguousarray(
            uw[:, e_core, :, hh * 1024:(hh + 1) * 1024] * fnw[:, :, None]))
        downw = _pmaj(np.ascontiguousarray(dw[:, e_core, hh * 1024:(hh + 1) * 1024]))
        selw = np.zeros((L, E, 128), f32)
        for l in range(L):
            selw[l, e_core, :] = 1.0 / np.sqrt(l + 1)
        headw = _pmaj(np.ascontiguousarray(
            (tok_emb[c * VS:(c + 1) * VS] * lnf[None, :]).T))
        in_maps.append(dict(
            x0=x0, biasP=biasP, qkvw=qkvw, ow=ow_c, routw=routw,
            gatew=gatew, upw=upw, downw=downw, selw=selw, headw=headw,
            cones=cones, ident=ident))
    return in_maps


def kernel(**inputs):
    nc = _get_nc()
    in_maps = make_in_maps(**inputs)
    res = run_bass_kernel_spmd(nc, in_maps, list(range(NC_N)))
    logits = np.concatenate([res.results[c]["logits"] for c in range(NC_N)],
                            axis=1)
    return logits.reshape(B, T, V)


# revision 26
# speedup vs baseline: 1.1124x; 1.0508x over previous
"""Trainium2 Bass kernel for a 2-layer MoE GPT (moe_routing).

Model: tok_emb lookup -> 2x [RMSNorm -> causal ALiBi attention -> RMSNorm ->
top-2-of-4 MoE (dense expert compute, sparse combine)] -> RMSNorm -> tied head.

Sharding over 8 NeuronCores:
- attention: head-parallel (2 of 16 heads per core), o_proj partial-summed
  via AllReduce.
- MoE: expert-parallel (expert e=c//2, hidden half c%2 per core), partial
  down_proj outputs AllReduced (routing weights folded in pre-reduce).
- LM head: vocab-parallel (4000 rows per core), concatenated on host.

v2 structure (vs the v1 baseline):
- AllReduces split per 512-token half and pipelined with compute; the
  residual x/8 is folded into each core's AR payload so the AR output IS
  the new residual (DMA'd straight into x, no post-AR adds).
- RMS scaling folded into matmul evacuation for attention qkv and the LM
  head (linear paths), so those matmuls don't wait on the norm.
- Softmax denominator via an augmented ones-column in the value tiles
  (row 64 of the AV accumulation), no separate denominator matmuls.
- Norm squares on the Scalar engine; fused scale-add evacuations
  (scalar_tensor_tensor) for AR payloads.
- Host-side pre-transposed [128, chunk, free] layouts -> single dense
  DMAs for activations, weights and AR buffers.
"""

import sys

sys.path.insert(0, "/opt/trn_rl_repo")

import numpy as np

import concourse.bass as bass  # noqa: E402
import concourse.bacc as bacc  # noqa: E402
import concourse.tile as tile  # noqa: E402
from concourse import mybir  # noqa: E402
from concourse.bass_utils import run_bass_kernel_spmd  # noqa: E402

dt = mybir.dt
F32 = dt.float32
F32R = dt.float32r
AF = mybir.ActivationFunctionType
ALU = mybir.AluOpType

V, D, H, HD, L, E, HID, TOPK, B, T = 32000, 1024, 16, 64, 2, 4, 2048, 2, 1, 1024
EPS = 1e-8
NC_N = 8
VS = V // NC_N            # vocab shard per core (4000)
VC = 500                  # head matmul free-dim chunk (8 * 500 = 4000)
DC = D // 128             # 8 partition chunks of the model dim
HC = 1024 // 128          # 8 hidden chunks of the per-core expert slice
NQ = 2                    # T halves of 512 for matmul free dim
TQ = T // NQ              # 512
TC2 = DC // NQ            # 128-token chunks per half (4)
NEG = -1.0e30

# (nq, kc) score tiles that are not fully causally masked, in emission order
ATT_TILES = [(nq, kc) for nq in range(NQ) for kc in range((nq + 1) * 4)]


def _ap(x):
    return x.ap() if hasattr(x, "ap") else x


def build_nc():
    nc = bacc.Bacc("TRN2", target_bir_lowering=False, debug=False,
                   num_devices=NC_N)

    # ---- DRAM I/O (per-core payloads; same shapes on every core) ----
    # activations / weights pre-transposed host-side to [128, chunk, free]
    x0 = nc.dram_tensor("x0", [128, DC, T], F32, kind="ExternalInput")
    biasP = nc.dram_tensor("biasP", [2 * len(ATT_TILES), 128, TQ], F32,
                           kind="ExternalInput")
    qkvw = nc.dram_tensor("qkvw", [L, 128, DC, 384], F32, kind="ExternalInput")
    ow = nc.dram_tensor("ow", [L, 128, DC, D], F32, kind="ExternalInput")
    routw = nc.dram_tensor("routw", [L, 128, DC, E], F32, kind="ExternalInput")
    gatew = nc.dram_tensor("gatew", [L, 128, DC, 1024], F32, kind="ExternalInput")
    upw = nc.dram_tensor("upw", [L, 128, DC, 1024], F32, kind="ExternalInput")
    downw = nc.dram_tensor("downw", [L, 128, HC, D], F32, kind="ExternalInput")
    selw = nc.dram_tensor("selw", [L, E, 128], F32, kind="ExternalInput")
    headw = nc.dram_tensor("headw", [128, DC, VS], F32, kind="ExternalInput")
    cones = nc.dram_tensor("cones", [128, 128], F32, kind="ExternalInput")
    ident = nc.dram_tensor("ident", [128, 128], F32, kind="ExternalInput")
    logits = nc.dram_tensor("logits", [T, VS], F32, kind="ExternalOutput")

    import os
    repeat = int(os.environ.get("KERNEL_REPEAT", "1"))
    with tile.TileContext(nc) as tc, nc.allow_low_precision(
            reason="float32r tiles store full fp32 bits; PE truncates on read"):
        for _ in range(repeat):
            _emit(nc, tc, x0, biasP, qkvw, ow, routw, gatew, upw, downw, selw,
                  headw, cones, ident, logits)
    nc.compile()
    return nc


def _emit(nc, tc, x0, biasP, qkvw, ow, routw, gatew, upw, downw, selw,
          headw, cones, ident, logits):
    import os
    SKIP_ATTN = os.environ.get("SKIP_ATTN") == "1"
    SKIP_MOE = os.environ.get("SKIP_MOE") == "1"
    SKIP_HEAD = os.environ.get("SKIP_HEAD") == "1"
    SKIP_AR = os.environ.get("SKIP_AR") == "1"
    ctxpools = []
    cm_of = {}

    def pool(name, bufs=1, space="SBUF"):
        p = tc.tile_pool(name=name, bufs=bufs, space=space)
        pm = p.__enter__()
        ctxpools.append(pm)
        cm_of[id(pm)] = p
        return pm

    def close_pool(pm):
        cm_of[id(pm)].__exit__(None, None, None)
        ctxpools.remove(pm)

    outer = pool("outer", bufs=1)
    dram = pool("dram", bufs=1, space="DRAM")

    # constants
    ones_t = outer.tile([128, 128], F32R, name="ones", tag="ones", bufs=1)
    nc.sync.dma_start(out=ones_t[:], in_=_ap(cones)[:].bitcast(F32R))
    id_t = outer.tile([128, 128], F32, name="ident", tag="ident", bufs=1)
    nc.sync.dma_start(out=id_t[:], in_=_ap(ident)[:])

    # residual stream [128, DC, T] fp32 (chunk a holds rows a*128..a*128+127)
    x_all = outer.tile([128, DC, T], F32R, name="x", tag="x", bufs=1)
    nc.sync.dma_start(out=x_all[:], in_=_ap(x0)[:].bitcast(F32R))
    xr = x_all[:]  # matmul-operand view

    # normalized activations for the MoE (fp32 bits, tagged f32r)
    xn_all = outer.tile([128, DC, T], F32R, name="xn", tag="xn", bufs=1)

    # staging tiles for AR payload chunks (triple-buffered; one DMA per
    # 128-row chunk)
    p_stage = pool("stage", bufs=3)

    def ts_(h):
        return slice(h * TQ, (h + 1) * TQ)

    def rstd_half(tag, h, rstdb, rstd_out=None):
        """rstdb[:, :] = broadcast of 1/sqrt(mean_d x^2 + EPS) for half h."""
        ts = ts_(h)
        p_sq = pool(f"sq_{tag}", bufs=1)
        p_ps = pool(f"nps_{tag}", bufs=1, space="PSUM")
        ssq = p_ps.tile([1, TQ], F32, name="ssq", tag="ssq", bufs=1)
        for i in range(DC):
            sq = p_sq.tile([128, TQ], F32R, name="sq", tag="sq", bufs=2)
            nc.scalar.activation(sq[:], x_all[:, i, ts], AF.Square)
            nc.tensor.matmul(ssq[:], ones_t[:, 0:1], sq[:],
                             start=(i == 0), stop=(i == DC - 1))
        tmp = p_sq.tile([1, 2, TQ], F32, name="ntmp", tag="ntmp", bufs=1)
        t0 = tmp[:, 0, :]
        t1 = tmp[:, 1, :]
        nc.vector.tensor_scalar(t0, ssq[:], 1.0 / D, EPS, ALU.mult, ALU.add)
        nc.scalar.activation(t1, t0, AF.Sqrt)
        if rstd_out is None:
            rstd_out = p_sq.tile([1, TQ], F32R, name="rstd", tag="rstd", bufs=1)
        nc.vector.reciprocal(rstd_out[:], t1)
        if rstdb is not None:
            bps = p_ps.tile([128, TQ], F32, name="bps", tag="bps", bufs=1)
            nc.tensor.matmul(bps[:], ones_t[0:1, :], rstd_out[:],
                             start=True, stop=True)
            nc.vector.tensor_copy(rstdb[:], bps[:])
        close_pool(p_ps)
        close_pool(p_sq)

    # qkv weights for BOTH layers preloaded (small, 12KB/partition each) so
    # next-layer attention can start under the previous layer's MoE AR
    qkv_l = []
    for l in range(L):
        t = outer.tile([128, DC, 384], F32R, name=f"qkvw{l}", tag=f"qkvw{l}",
                       bufs=1)
        nc.sync.dma_start(out=t[:], in_=_ap(qkvw)[l].bitcast(F32R))
        qkv_l.append(t)

    # ---------------- layers ----------------
    for l in range(L):
        if SKIP_ATTN:
            break
        # ---- attention ----
        p_aw = pool(f"aw{l}", bufs=1)
        qkv_t = qkv_l[l]
        # full o_proj weight (all 16 heads): with the y-AllGather every core
        # computes the complete o_proj locally (no attention AllReduce)
        ow_t = p_aw.tile([128, DC, D], F32R, name="ow", tag="ow", bufs=1)
        nc.sync.dma_start(out=ow_t[:], in_=_ap(ow)[l].bitcast(F32R))

        rstdb_a = [p_aw.tile([128, TQ], F32, name=f"rsb{h}", tag=f"rsb{h}",
                             bufs=1) for h in range(NQ)]
        for h in range(NQ):
            rstd_half(f"a{l}h{h}", h, rstdb_a[h])

        # qkv on raw x; rstd folded into the PSUM evacuation
        qkvT = p_aw.tile([128, 3, T], F32R, name="qkvT", tag="qkvT", bufs=1)
        p_qps = pool(f"qps{l}", bufs=1, space="PSUM")
        for nh in range(NQ):
            ts = ts_(nh)
            for m in range(3):
                ps = p_qps.tile([128, TQ], F32, name="qkv", tag="qkv", bufs=3)
                for i in range(DC):
                    nc.tensor.matmul(ps[:],
                                     qkv_t[:, i, m * 128:(m + 1) * 128],
                                     xr[:, i, ts],
                                     start=(i == 0), stop=(i == DC - 1))
                nc.vector.tensor_tensor(
                    qkvT[:, m, ts], ps[:], rstdb_a[nh][:], ALU.mult)
        qT = qkvT[:, 0, :]
        kT = qkvT[:, 1, :]
        vT = qkvT[:, 2, :]
        # v in natural [token, hd] layout, augmented with a ones column per
        # head (row 64 of the AV output then accumulates the softmax denom)
        v_aug = p_aw.tile([128, DC, 130], F32R, name="vaug", tag="vaug", bufs=1)
        nc.vector.tensor_copy(v_aug[:, :, 64:65], ones_t[:, 0:DC].unsqueeze(2))
        nc.vector.tensor_copy(v_aug[:, :, 129:130], ones_t[:, 0:DC].unsqueeze(2))
        for tcn in range(DC):
            tp = p_qps.tile([128, 128], F32, name="vt", tag="vt", bufs=2)
            nc.tensor.transpose(tp[:],
                                vT[:, tcn * 128:(tcn + 1) * 128].bitcast(F32),
                                id_t[:])
            for h in range(2):
                nc.vector.tensor_copy(v_aug[:, tcn, h * 65:h * 65 + 64],
                                      tp[:, h * 64:(h + 1) * 64])
        close_pool(p_qps)

        p_sps = pool(f"sps{l}", bufs=1, space="PSUM")
        p_sc = pool(f"sc{l}", bufs=1)
        p_ops = pool(f"ops{l}", bufs=1, space="PSUM")
        for nq in range(NQ):
            ts = ts_(nq)
            kcs = [kc for (q, kc) in ATT_TILES if q == nq]
            yps = [p_sps.tile([65, TQ], F32, name=f"yt{h}", tag=f"yt{h}", bufs=1)
                   for h in range(2)]
            for ki, kc in enumerate(kcs):
                for h in range(2):
                    hp = h * 64
                    st = p_sps.tile([128, TQ], F32, name="st", tag="st", bufs=3)
                    nc.tensor.matmul(st[:],
                                     kT[hp:hp + 64, kc * 128:(kc + 1) * 128],
                                     qT[hp:hp + 64, ts],
                                     start=True, stop=True)
                    bti = (h * len(ATT_TILES)
                           + ATT_TILES.index((nq, kc)))
                    bt = p_sc.tile([128, TQ], F32, name="bias", tag="bias", bufs=3)
                    nc.sync.dma_start(out=bt[:], in_=_ap(biasP)[bti])
                    es = p_sc.tile([128, TQ], F32R, name="es", tag="es", bufs=3)
                    nc.vector.tensor_tensor(es[:], st[:], bt[:], ALU.add)
                    nc.scalar.activation(es[:], es[:], AF.Exp)
                    nc.tensor.matmul(yps[h][:, :],
                                     v_aug[:, kc, h * 65:(h + 1) * 65],
                                     es[:],
                                     start=(ki == 0), stop=(ki == len(kcs) - 1))
            # normalize and AllGather y across cores (heads 2c, 2c+1 land at
            # partition block c of the gathered [1024, TQ])
            ariny = dram.tile([128, TQ], F32, name=f"ariny{l}{nq}",
                              tag=f"ariny{nq}", bufs=1)
            arouty = dram.tile([DC, 128, TQ], F32, name=f"arouty{l}{nq}",
                               tag=f"arouty{nq}", bufs=1, addr_space="Shared")
            y2 = p_sc.tile([128, TQ], F32, name="y2", tag="y2", bufs=2)
            for h in range(2):
                rc = p_sc.tile([1, TQ], F32R, name="rc", tag="rc", bufs=2)
                nc.vector.reciprocal(rc[:], yps[h][64:65, :])
                rps = p_sps.tile([64, TQ], F32, name="rb", tag="rb", bufs=1)
                nc.tensor.matmul(rps[:], ones_t[0:1, 0:64],
                                 rc[:], start=True, stop=True)
                rsb = p_sc.tile([64, TQ], F32, name="rsb", tag="rsb", bufs=2)
                nc.scalar.copy(rsb[:], rps[:])
                nc.vector.tensor_tensor(
                    y2[h * 64:(h + 1) * 64, :], yps[h][0:64, :], rsb[:, :],
                    ALU.mult)
            nc.sync.dma_start(out=ariny[:], in_=y2[:])
            nc.gpsimd.collective_compute(
                "AllGather", ALU.bypass, replica_groups=[list(range(NC_N))],
                ins=[ariny.opt()], outs=[arouty.opt()])
            y_nat = p_sc.tile([128, DC, TQ], F32R, name="ynat", tag="ynat",
                              bufs=2)
            # gpsimd-queue readback: only collectives live there, so the
            # wait on the AG sem cannot stall compute or other DMA issue
            nc.gpsimd.dma_start(
                out=y_nat[:],
                in_=arouty[:].rearrange("a p m -> p a m").bitcast(F32R))
            # full local o_proj + in-place residual add
            for i in range(DC):
                ps = p_ops.tile([128, TQ], F32, name="o", tag="o", bufs=2)
                for rc_ in range(DC):
                    nc.tensor.matmul(ps[:],
                                     ow_t[:, rc_, i * 128:(i + 1) * 128],
                                     y_nat[:, rc_, :],
                                     start=(rc_ == 0), stop=(rc_ == DC - 1))
                nc.vector.tensor_tensor(x_all[:, i, ts], x_all[:, i, ts],
                                        ps[:], ALU.add)
        for p in (p_ops, p_sc, p_sps, p_aw):
            close_pool(p)

        if SKIP_MOE:
            continue
        # ---- MoE ----
        p_mw = pool(f"mw{l}", bufs=1)
        p_msc = pool(f"msc{l}", bufs=1)
        rout_t = p_mw.tile([128, DC, E], F32, name="routw", tag="routw", bufs=1)
        nc.sync.dma_start(out=rout_t[:], in_=_ap(routw)[l])
        sel_t = p_mw.tile([E, 128], F32, name="sel", tag="sel", bufs=1)
        nc.sync.dma_start(out=sel_t[:], in_=_ap(selw)[l])
        gate_t = p_mw.tile([128, DC, 1024], F32R, name="gate", tag="gate", bufs=1)
        nc.sync.dma_start(out=gate_t[:], in_=_ap(gatew)[l].bitcast(F32R))
        up_t = p_mw.tile([128, DC, 1024], F32R, name="up", tag="up", bufs=1)
        nc.sync.dma_start(out=up_t[:], in_=_ap(upw)[l].bitcast(F32R))

        for mh in range(NQ):
            ts = ts_(mh)
            rstdb_m = p_msc.tile([128, TQ], F32, name="rsbm", tag="rsbm", bufs=2)
            rstd_half(f"f{l}h{mh}", mh, rstdb_m)
            for i in range(DC):
                nc.vector.tensor_tensor(xn_all[:, i, ts],
                                        x_all[:, i, ts],
                                        rstdb_m[:], ALU.mult)

            # router in true fp32 on token chunks of this half
            p_rps = pool(f"rps{l}{mh}", bufs=1, space="PSUM")
            rlog = p_msc.tile([128, TC2, E], F32, name="rlog", tag="rlog", bufs=2)
            for tj in range(TC2):
                tcn = mh * TC2 + tj
                ps = p_rps.tile([128, E], F32, name="rl", tag="rl", bufs=2)
                for i in range(DC):
                    nc.tensor.matmul(ps[:],
                                     xn_all[:, i, tcn * 128:(tcn + 1) * 128]
                                     .bitcast(F32),
                                     rout_t[:, i, :],
                                     start=(i == 0), stop=(i == DC - 1))
                nc.vector.tensor_copy(rlog[:, tj, :], ps[:])
            # batched top-2 -> combine weights  [128, (tj, e)]
            m1 = p_msc.tile([128, TC2], F32, name="m1", tag="m1", bufs=2)
            nc.vector.tensor_reduce(m1[:], rlog[:], mybir.AxisListType.X, ALU.max)
            eq1 = p_msc.tile([128, TC2, E], F32, name="eq1", tag="eq1", bufs=2)
            nc.vector.tensor_tensor(eq1[:], rlog[:],
                                    m1[:].unsqueeze(2).broadcast_to([128, TC2, E]),
                                    ALU.is_equal)
            msk = p_msc.tile([128, TC2, E], F32, name="msk", tag="msk", bufs=2)
            nc.vector.scalar_tensor_tensor(msk[:], eq1[:], NEG, rlog[:],
                                           ALU.mult, ALU.add)
            m2 = p_msc.tile([128, TC2], F32, name="m2", tag="m2", bufs=2)
            nc.vector.tensor_reduce(m2[:], msk[:], mybir.AxisListType.X, ALU.max)
            eq2 = p_msc.tile([128, TC2, E], F32, name="eq2", tag="eq2", bufs=2)
            nc.vector.tensor_tensor(eq2[:], msk[:],
                                    m2[:].unsqueeze(2).broadcast_to([128, TC2, E]),
                                    ALU.is_equal)
            d12 = p_msc.tile([128, TC2], F32, name="d12", tag="d12", bufs=2)
            nc.vector.tensor_tensor(d12[:], m1[:], m2[:], ALU.subtract)
            w1 = p_msc.tile([128, TC2], F32, name="w1", tag="w1", bufs=2)
            nc.scalar.activation(w1[:], d12[:], AF.Sigmoid)
            w2 = p_msc.tile([128, TC2], F32, name="w2", tag="w2", bufs=2)
            nc.vector.tensor_scalar(w2[:], w1[:], -1.0, 1.0, ALU.mult, ALU.add)
            comb = p_msc.tile([128, TC2, E], F32, name="comb", tag="comb", bufs=2)
            nc.vector.tensor_tensor(comb[:], eq1[:],
                                    w1[:].unsqueeze(2).broadcast_to([128, TC2, E]),
                                    ALU.mult)
            eq2w = p_msc.tile([128, TC2, E], F32, name="eq2w", tag="eq2w", bufs=2)
            nc.vector.tensor_tensor(eq2w[:], eq2[:],
                                    w2[:].unsqueeze(2).broadcast_to([128, TC2, E]),
                                    ALU.mult)
            nc.vector.tensor_tensor(comb[:], comb[:], eq2w[:], ALU.add)
            # per-token scaled combine weight for this core's expert,
            # broadcast over 128 partitions: bc[p, t] = sel.T @ comb_tj.T
            bc = p_msc.tile([128, TQ], F32, name="bc", tag="bc", bufs=2)
            for tj in range(TC2):
                tp = p_rps.tile([E, 128], F32, name="ct", tag="ct", bufs=2)
                nc.tensor.transpose(tp[:], comb[:, tj, :], id_t[:])
                ct = p_msc.tile([E, 128], F32, name="cts", tag="cts", bufs=2)
                nc.vector.tensor_copy(ct[:], tp[:])
                bp = p_rps.tile([128, 128], F32, name="bcp", tag="bcp", bufs=2)
                nc.tensor.matmul(bp[:], sel_t[:], ct[:], start=True, stop=True)
                nc.vector.tensor_copy(bc[:, tj * 128:(tj + 1) * 128], bp[:])
            close_pool(p_rps)

            # experts: gate/up -> silu*up*bc -> down; residual folded at
            # evacuation; AR per half
            arin2 = dram.tile([128, DC, TQ], F32, name=f"arin_m{l}{mh}",
                              tag=f"arin_m{mh}", bufs=1)
            arout2 = dram.tile([128, DC, TQ], F32, name=f"arout_m{l}{mh}",
                               tag=f"arout_m{mh}", bufs=1, addr_space="Shared")
            p_mps = pool(f"mps{l}{mh}", bufs=1, space="PSUM")
            gu_all = p_msc.tile([128, HC, TQ], F32R, name="gu", tag="gu", bufs=1)
            for hc in range(HC):
                gps = p_mps.tile([128, TQ], F32, name="g", tag="g", bufs=2)
                for i in range(DC):
                    nc.tensor.matmul(gps[:], gate_t[:, i, hc * 128:(hc + 1) * 128],
                                     xn_all[:, i, ts],
                                     start=(i == 0), stop=(i == DC - 1))
                gs = p_msc.tile([128, TQ], F32, name="gs", tag="gs", bufs=2)
                nc.scalar.activation(gs[:], gps[:], AF.Silu)
                ups = p_mps.tile([128, TQ], F32, name="u", tag="u", bufs=2)
                for i in range(DC):
                    nc.tensor.matmul(ups[:], up_t[:, i, hc * 128:(hc + 1) * 128],
                                     xn_all[:, i, ts],
                                     start=(i == 0), stop=(i == DC - 1))
                gu = gu_all[:, hc, :]
                nc.vector.tensor_tensor(gu, gs[:], ups[:], ALU.mult)
                nc.vector.tensor_tensor(gu, gu, bc[:, :], ALU.mult)
            for i in range(DC):
                dw = p_mw.tile([128, HC, 128], F32R, name="down", tag="down", bufs=2)
                nc.sync.dma_start(
                    out=dw[:],
                    in_=_ap(downw)[l][:, :, i * 128:(i + 1) * 128].bitcast(F32R))
                dps = p_mps.tile([128, TQ], F32, name="d", tag="d", bufs=2)
                for hc in range(HC):
                    nc.tensor.matmul(dps[:], dw[:, hc, :], gu_all[:, hc, :],
                                     start=(hc == 0), stop=(hc == HC - 1))
                stg = p_stage.tile([128, TQ], F32, name="stg", tag="stg", bufs=3)
                nc.vector.scalar_tensor_tensor(
                    stg[:], x_all[:, i, ts].bitcast(F32), 1.0 / NC_N, dps[:],
                    ALU.mult, ALU.add)
                nc.sync.dma_start(out=arin2[:, i, :], in_=stg[:])
            close_pool(p_mps)
            if not SKIP_AR:
                nc.gpsimd.collective_compute(
                    "AllReduce", ALU.add, replica_groups=[list(range(NC_N))],
                    ins=[arin2.opt()], outs=[arout2.opt()])
                nc.gpsimd.dma_start(out=x_all[:, :, ts],
                                    in_=arout2[:].bitcast(F32R))
        for p in (p_msc, p_mw):
            close_pool(p)

    if SKIP_HEAD:
        for pm in reversed(list(ctxpools)):
            close_pool(pm)
        return
    # ---- final norm + vocab-sharded tied head ----
    # rstd folded into the head evacuation (per-token scale on partitions)
    p_hw = pool("hw", bufs=1)
    p_hps = pool("hps", bufs=1, space="PSUM")
    rstdc = p_hw.tile([128, DC], F32, name="rstdc", tag="rstdc", bufs=1)
    for h in range(NQ):
        rstdb_h = p_hw.tile([128, TQ], F32, name="rsbh", tag="rsbh", bufs=2)
        rstd_half(f"hd{h}", h, rstdb_h)
        # per-token rstd as a column vector per 128-token chunk: transpose
        # a [128, 128] window of the row-broadcast tile; its columns are
        # all the per-token column we need
        for tj in range(TC2):
            tcn = h * TC2 + tj
            cp = p_hps.tile([128, 128], F32, name="rc", tag="rch", bufs=2)
            nc.tensor.transpose(cp[:],
                                rstdb_h[:, tj * 128:(tj + 1) * 128], id_t[:])
            nc.vector.tensor_copy(rstdc[:, tcn:tcn + 1], cp[:, 0:1])
    for vc in range(VS // VC):
        hw = p_hw.tile([128, DC, VC], F32R, name="hw", tag="hw", bufs=2)
        nc.sync.dma_start(
            out=hw[:],
            in_=_ap(headw)[:, :, vc * VC:(vc + 1) * VC].bitcast(F32R))
        for tcn in range(DC):
            ps = p_hps.tile([128, VC], F32, name="h", tag="h", bufs=4)
            for i in range(DC):
                nc.tensor.matmul(ps[:],
                                 xr[:, i, tcn * 128:(tcn + 1) * 128],
                                 hw[:, i, :],
                                 start=(i == 0), stop=(i == DC - 1))
            lg = p_hw.tile([128, VC], F32, name="lg", tag="lg", bufs=4)
            nc.scalar.activation(lg[:], ps[:], AF.Copy,
                                 scale=rstdc[:, tcn:tcn + 1])
            nc.sync.dma_start(
                out=_ap(logits)[tcn * 128:(tcn + 1) * 128,
                                vc * VC:(vc + 1) * VC],
                in_=lg[:])

    for pm in reversed(list(ctxpools)):
        close_pool(pm)


_NC_CACHE = None


def _get_nc():
    global _NC_CACHE
    if _NC_CACHE is None:
        _NC_CACHE = build_nc()
    return _NC_CACHE


def _pmaj(a):
    """[.., D_outer, free] with D_outer = 128*DC -> [.., 128, DC, free]."""
    s = a.shape
    d = s[-2]
    a = a.reshape(*s[:-2], d // 128, 128, s[-1])
    order = list(range(a.ndim))
    order[-3], order[-2] = order[-2], order[-3]
    return np.ascontiguousarray(a.transpose(order))


def make_in_maps(idx, tok_emb, attn_norm_w, q_w, q_b, kv_w, kv_b, o_w, o_b,
                 ffn_norm_w, router_w, gate_w, up_w, down_w, lnf_w):
    """Host-side sharding: build the per-core input dicts."""
    f32 = np.float32
    idx = np.asarray(idx)
    tok_emb = np.asarray(tok_emb, f32)
    x0T = np.ascontiguousarray(tok_emb[idx[0]].T)  # [D, T]
    x0 = _pmaj(x0T)

    qw = np.asarray(q_w, f32).reshape(L, D, H, HD)
    kvw = np.asarray(kv_w, f32).reshape(L, D, 2, H, HD)
    owf = np.asarray(o_w, f32).reshape(L, H, HD, D)
    anw = np.asarray(attn_norm_w, f32)
    fnw = np.asarray(ffn_norm_w, f32)
    rw = np.asarray(router_w, f32)
    gw = np.asarray(gate_w, f32)
    uw = np.asarray(up_w, f32)
    dw = np.asarray(down_w, f32)
    lnf = np.asarray(lnf_w, f32)

    cones = np.ones((128, 128), f32)
    ident = np.eye(128, dtype=f32)

    in_maps = []
    for c in range(NC_N):
        h0 = 2 * c
        e_core, hh = c // 2, c % 2
        # attention bias tiles (alibi + causal), valid tiles only
        nbt = len(ATT_TILES)
        biasP = np.empty((2 * nbt, 128, TQ), f32)
        for hi in range(2):
            slope = (h0 + hi + 1) / H
            for ti, (nq, kc) in enumerate(ATT_TILES):
                k = kc * 128 + np.arange(128, dtype=f32)[:, None]
                q = (nq * TQ + np.arange(TQ, dtype=f32))[None, :]
                b = slope * (k - q)
                b[k > q] = NEG
                biasP[hi * nbt + ti] = b
        # qkv weights: attn_norm folded in, q scaled by 1/sqrt(HD)
        qkvw = np.empty((L, D, 384), f32)
        for l in range(L):
            sc = anw[l][:, None]
            qkvw[l, :, 0:128] = (
                qw[l][:, h0:h0 + 2].reshape(D, 128) * sc / np.sqrt(HD))
            qkvw[l, :, 128:256] = kvw[l][:, 0, h0:h0 + 2].reshape(D, 128) * sc
            qkvw[l, :, 256:384] = kvw[l][:, 1, h0:h0 + 2].reshape(D, 128) * sc
        qkvw = _pmaj(qkvw)
        ow_c = _pmaj(owf.reshape(L, H * HD, D))
        routw = _pmaj(rw * fnw[:, :, None])
        gatew = _pmaj(np.ascontiguousarray(
            gw[:, e_core, :, hh * 1024:(hh + 1) * 1024] * fnw[:, :, None]))
        upw = _pmaj(np.ascontiguousarray(
            uw[:, e_core, :, hh * 1024:(hh + 1) * 1024] * fnw[:, :, None]))
        downw = _pmaj(np.ascontiguousarray(dw[:, e_core, hh * 1024:(hh + 1) * 1024]))
        selw = np.zeros((L, E, 128), f32)
        for l in range(L):
            selw[l, e_core, :] = 1.0 / np.sqrt(l + 1)
        headw = _pmaj(np.ascontiguousarray(
            (tok_emb[c * VS:(c + 1) * VS] * lnf[None, :]).T))
        in_maps.append(dict(
            x0=x0, biasP=biasP, qkvw=qkvw, ow=ow_c, routw=routw,
            gatew=gatew, upw=upw, downw=downw, selw=selw, headw=headw,
            cones=cones, ident=ident))
    return in_maps


def kernel(**inputs):
    nc = _get_nc()
    in_maps = make_in_maps(**inputs)
    res = run_bass_kernel_spmd(nc, in_maps, list(range(NC_N)))
    logits = np.concatenate([res.results[c]["logits"] for c in range(NC_N)],
                            axis=1)
    return logits.reshape(B, T, V)
